# revision 7
# baseline (speedup 1.0000x reference)
"""Distributed Trainium2 kernel for the AttentionBlock problem.

Full inputs:
  x     [4, 2048, 512] f32
  w_qkv [512, 1536]    f32   (columns: q | k | v, each 512 wide)
  w_out [512, 512]     f32
  b_out [512]          f32

Sharding over 8 cores: core c handles batch (c // 2) and head-group
(c % 2) -> 4 heads of 64 dims each (feature slice of 256 per section).
Each core computes a partial output projection (its 4 heads' contribution
to out = attn @ w_out); the host sums the two partials per batch and adds
the bias.

Per-core dataflow (bf16 matmuls, all intermediates in SBUF; x and the
weights are converted to bf16 on the host so no on-device casts and half
the input DMA traffic):
  xT [512, 2048] -> qT,kT [256+256, 2048] (transposed layout)
                 -> v natural [2048, 4*65] (65th col per head = ones,
                    which makes the softmax denominator fall out of the
                    PV matmul as output row 64 for free: the PV stream
                    time is N cycles regardless of the stationary width)
  per (query-block of 512, head-pair) block, 16 j-steps:
    S^T[j 128, i 512x2] = kT.T @ qT   (two heads in concurrent 64-row
                                       PE tiles -> one [128,1024] PSUM)
    P^T = exp(S^T * 0.125)            (ScalarE -> SBUF bf16)
    outT[f 65, i 512] += v'.T @ P^T   (j-major PSUM accumulation)
    attnT[f, i] = outT[0:64] * (1/l)  (DVE recip from PSUM + gpsimd
                                       broadcast + DVE mul)
  out_partial[i 128, 512] = attnT.T @ w_out_shard -> DRAM

Schedule: ScalarE (128 exps of [128,1024], ~136us busy) is the critical
engine; everything else hides under it. Key tricks:
  - PV(j) is issued one slot late so its matmuls never wait on the
    just-finished exp (no PE pipe-refill stall).
  - The remaining q/k/v projections and the output projections are
    injected between attention slots where the PE has slack; the first
    two blocks are interleaved in j-halves (b0a, b1a, b0b, b1b) to
    spread the v-projection injections over 32 slots.
  - Normalizations run overlapped with the next block; the tail after
    the last exp is only PV flush + 2 norms + 4 output-projection
    chunks.
  - A few warmup matmuls run during the DMA ramp so the PE HAM clock
    gate is already at full rate when the real matmuls arrive.
"""

import sys

if "/opt/trn_rl_repo" not in sys.path:
    sys.path.insert(0, "/opt/trn_rl_repo")

import numpy as np

DIM = 512
HEADS = 8
DIM_HEAD = 64
INNER = 512
B, N = 4, 2048
N_CORES = 8
HEADS_PER_CORE = 4
FEAT = HEADS_PER_CORE * DIM_HEAD  # 256 features per core per section
SCALE = DIM_HEAD ** -0.5  # 0.125

N_JB = N // 128  # 16 j-blocks
N_WARMUP_MM = 4

_CACHED = {}


def _build():
    import concourse.mybir as mybir
    import concourse.tile as tile
    from concourse import bacc

    f32 = mybir.dt.float32
    bf16 = mybir.dt.bfloat16
    EXP = mybir.ActivationFunctionType.Exp
    MUL = mybir.AluOpType.mult

    nc = bacc.Bacc("TRN2", target_bir_lowering=False, debug=False,
                   num_devices=N_CORES)

    xT = nc.declare_dram_parameter("xT", [DIM, N], bf16, isOutput=False)
    wqk = nc.declare_dram_parameter("wqk", [DIM, 2 * FEAT], bf16,
                                    isOutput=False)
    wv = nc.declare_dram_parameter("wv", [DIM, FEAT], bf16, isOutput=False)
    w_out = nc.declare_dram_parameter("w_out", [FEAT, DIM], bf16,
                                      isOutput=False)
    out = nc.declare_dram_parameter("out", [N, DIM], f32, isOutput=True)

    with tile.TileContext(nc) as tc:
        with (
            tc.tile_pool(name="xb", bufs=1) as xb_pool,
            tc.tile_pool(name="wq", bufs=1) as w_pool,
            tc.tile_pool(name="qkt", bufs=1) as qkt_pool,
            tc.tile_pool(name="vs", bufs=1) as v_pool,
            tc.tile_pool(name="pt", bufs=8) as pt_pool,
            tc.tile_pool(name="attnT", bufs=1) as attnT_pool,
            tc.tile_pool(name="scl", bufs=8) as scl_pool,
            tc.tile_pool(name="dout", bufs=4) as dout_pool,
            tc.tile_pool(name="warm", bufs=1) as warm_pool,
            tc.tile_pool(name="mm", bufs=2, space="PSUM") as mm_psum,
            tc.tile_pool(name="pv", bufs=4, space="PSUM") as pv_psum,
        ):
            # ---- PE warmup: keep the HAM clock gate busy during the DMA
            # ramp so real matmuls start at full rate ----
            wsrc = warm_pool.tile([128, 512], bf16, tag="wsrc", name="wsrc")
            nc.vector.memset(wsrc[:], 0.0)
            wps = mm_psum.tile([128, 512], f32, tag="qs", name="warmps")
            with nc.named_scope("warmup"):
                for _ in range(N_WARMUP_MM):
                    nc.tensor.matmul(wps[:], wsrc[:, 0:128], wsrc[:],
                                     start=True, stop=True)

            # ---- batched DMAs (one per x column-block / weight), issued
            # in need order: x block 0, wqk, wv, rest of x, wo ----
            xT_r = xT.rearrange("(k p) n -> p k n", k=4)
            wqk_r = wqk.rearrange("(k p) f -> p k f", k=4)
            wv_r = wv.rearrange("(k p) f -> p k f", k=4)
            wo_r = w_out.rearrange("(t p) f -> p t f", t=2)

            xb = [None] * 4  # [n] -> [128, 4, 512] bf16 (k in dim 1)

            def dma_x(n):
                t = xb_pool.tile([128, 4, 512], bf16, tag=f"xb{n}",
                                 name=f"xb{n}")
                nc.sync.dma_start(out=t[:],
                                  in_=xT_r[:, :, n * 512:(n + 1) * 512])
                xb[n] = t

            dma_x(0)
            wqk_t = w_pool.tile([128, 4, 512], bf16, tag="wqk", name="wqkb")
            nc.sync.dma_start(out=wqk_t[:], in_=wqk_r[:])
            wv_t = w_pool.tile([128, 4, FEAT], bf16, tag="wv", name="wvb")
            nc.sync.dma_start(out=wv_t[:], in_=wv_r[:])
            dma_x(1)
            dma_x(2)
            dma_x(3)
            wo = w_pool.tile([128, 2, DIM], bf16, tag="wo", name="wob")
            nc.sync.dma_start(out=wo[:], in_=wo_r[:])

            # ---- persistent SBUF tiles ----
            qkt = [qkt_pool.tile([128, N], bf16, tag=f"qkt{m}", name=f"qkt{m}")
                   for m in range(4)]
            vt = [v_pool.tile([128, 4 * 65], bf16, tag=f"v{j}", name=f"v{j}")
                  for j in range(N_JB)]
            attnT = [attnT_pool.tile([128, N], bf16, tag=f"aT{t}",
                                     name=f"aT{t}")
                     for t in range(2)]

            def proj_qk(m, n):
                ncol = slice(n * 512, (n + 1) * 512)
                with nc.named_scope("proj"):
                    ps = mm_psum.tile([128, 512], f32, tag="qs", name="psb")
                    for k in range(4):
                        nc.tensor.matmul(
                            ps[:],
                            wqk_t[:, k, m * 128:(m + 1) * 128],
                            xb[n][:, k, :],
                            start=(k == 0), stop=(k == 3),
                        )
                    nc.vector.tensor_copy(qkt[m][:, ncol], ps[:])

            def proj_v(j):
                n, jj = j // 4, j % 4
                with nc.named_scope("proj"):
                    ps = mm_psum.tile([128, 256], f32, tag="qs", name="psv")
                    for k in range(4):
                        nc.tensor.matmul(
                            ps[:],
                            xb[n][:, k, jj * 128:(jj + 1) * 128],
                            wv_t[:, k, :],
                            start=(k == 0), stop=(k == 3),
                        )
                    v_view = vt[j].rearrange("p (h f) -> p h f", h=4)
                    nc.vector.tensor_copy(
                        v_view[:, :, 0:64], ps.rearrange("p (h f) -> p h f", h=4)
                    )
                    nc.vector.memset(v_view[:, :, 64:65], 1.0)

            def normalize(pair, hh, ic_ps, i0):
                with nc.named_scope("norm"):
                    # copy the denominator row to SBUF first: a PSUM-sourced
                    # reciprocal_approx_fast reads the wrong partition
                    lrow = scl_pool.tile([1, 512], f32, tag="lrow", name="lrow")
                    nc.vector.tensor_copy(lrow[:], ic_ps[64:65, :])
                    rl = scl_pool.tile([1, 512], f32, tag="rl", name="rl")
                    nc.vector.reciprocal_approx_fast(rl[:], lrow[:])
                    rlb = scl_pool.tile([64, 512], f32, tag="rlb", name="rlb")
                    nc.gpsimd.partition_broadcast(rlb[:], rl[:])
                    nc.vector.tensor_tensor(
                        attnT[pair][hh * 64:(hh + 1) * 64, i0:i0 + 512],
                        ic_ps[0:64, :], rlb[:], MUL,
                    )

            out_r = out.rearrange("(c p) f -> p c f", c=16)
            ot_cur = [None]

            def outproj_chunk(chunk):
                ic, c = chunk // 4, chunk % 4
                with nc.named_scope("outproj"):
                    ps = mm_psum.tile([128, 512], f32, tag="qs", name="psd")
                    for t in range(2):
                        nc.tensor.matmul(
                            ps[:],
                            attnT[t][:, chunk * 128:(chunk + 1) * 128],
                            wo[:, t, :],
                            start=(t == 0), stop=(t == 1),
                        )
                    if c == 0:
                        ot_cur[0] = dout_pool.tile([128, 4, 512], f32,
                                                   tag="ot", name=f"ot{ic}")
                    nc.vector.tensor_copy(ot_cur[0][:, c, :], ps[:])
                    if c == 3:
                        nc.sync.dma_start(
                            out=out_r[:, 4 * ic:4 * ic + 4, :],
                            in_=ot_cur[0][:])

            # pending PV thunk: PV(j) is issued one attention slot later so
            # its matmuls never wait on the exp that produced their P tile.
            pend = [None]

            def flush_pv():
                if pend[0] is not None:
                    pend[0]()
                    pend[0] = None

            def attn_phase(ic, pair, jlo, jhi, outp, inject):
                qt, kt = qkt[pair], qkt[2 + pair]
                i0 = ic * 512
                for j in range(jlo, jhi):
                    with nc.named_scope("attnS"):
                        qs = mm_psum.tile([128, 1024], f32, tag="qs",
                                          name="qs")
                        for hh in range(2):
                            nc.tensor.matmul(
                                qs[:, hh * 512:(hh + 1) * 512],
                                kt[hh * 64:(hh + 1) * 64,
                                   j * 128:(j + 1) * 128],
                                qt[hh * 64:(hh + 1) * 64, i0:i0 + 512],
                                start=True, stop=True,
                            )
                    with nc.named_scope("exp"):
                        p = pt_pool.tile([128, 1024], bf16, tag="pt",
                                         name="ptile")
                        nc.scalar.activation(p[:], qs[:], EXP, scale=SCALE)
                    flush_pv()

                    def pv(p=p, j=j):
                        with nc.named_scope("pv"):
                            for hh in range(2):
                                h = 2 * pair + hh
                                nc.tensor.matmul(
                                    outp[hh][0:65, :],
                                    vt[j][:, h * 65:(h + 1) * 65],
                                    p[:, hh * 512:(hh + 1) * 512],
                                    start=(j == 0), stop=(j == N_JB - 1),
                                )
                    pend[0] = pv
                    for thunk in inject.get(j, ()):
                        thunk()

            def new_outp(ic, pair):
                return [pv_psum.tile([128, 512], f32, tag="pv",
                                     name=f"o{ic}{pair}{hh}")
                        for hh in range(2)]

            def norm_thunks(ic, pair, outp):
                return [
                    (lambda hh=hh: normalize(pair, hh, outp[hh], ic * 512))
                    for hh in range(2)
                ]

            def merge_inject(inject, extra, jlo):
                inj = {j: list(v) for j, v in inject.items()}
                for i, t in enumerate(extra):
                    inj.setdefault(jlo + i, []).insert(0, t)
                return inj

            P = proj_qk
            V = proj_v

            # ---- prologue: minimum work before the first exp ----
            P(0, 0)
            P(2, 0)
            V(0)
            V(1)

            # ---- ic0: the two blocks interleaved in j-halves so the
            # remaining projections spread over 32 slots ----
            o00 = new_outp(0, 0)
            o01 = new_outp(0, 1)
            attn_phase(0, 0, 0, 8, o00, {
                1: [lambda: V(2)],
                2: [lambda: V(3)],
                3: [lambda: P(2, 1), lambda: V(4)],
                4: [lambda: V(5)],
                5: [lambda: V(6)],
                6: [lambda: V(7), lambda: P(1, 0)],
                7: [lambda: P(3, 0)],
            })
            attn_phase(0, 1, 0, 8, o01, {
                2: [lambda: P(2, 2)],
                3: [lambda: P(3, 1)],
                4: [lambda: V(8)],
                5: [lambda: V(9)],
                6: [lambda: V(10)],
                7: [lambda: V(11)],
            })
            attn_phase(0, 0, 8, 16, o00, {
                10: [lambda: P(2, 3)],
                11: [lambda: P(3, 2)],
                12: [lambda: V(12)],
                13: [lambda: V(13)],
                14: [lambda: V(14)],
                15: [lambda: V(15)],
            })
            attn_phase(0, 1, 8, 16, o01, merge_inject({
                10: [lambda: P(3, 3)],
                11: [lambda: P(0, 1)],
                12: [lambda: P(1, 1)],
                13: [lambda: P(0, 2)],
                14: [lambda: P(1, 2)],
                15: [lambda: P(0, 3)],
            }, norm_thunks(0, 0, o00), 8))
            carry = norm_thunks(0, 1, o01)

            # ---- ic1..ic3: plain blocks; the previous block's norms are
            # injected into the first slots (after the slot-0 PV flush) and
            # the previous ic's output projections once its norms landed ----
            for ic in range(1, 4):
                for pair in range(2):
                    o = new_outp(ic, pair)
                    inject = {}
                    if pair == 1:
                        inject = {2 + 4 * c: [
                            (lambda c=c: outproj_chunk(4 * (ic - 1) + c))]
                            for c in range(4)}
                    elif ic == 1:
                        inject = {2: [lambda: P(1, 3)]}
                    attn_phase(ic, pair, 0, N_JB, o,
                               merge_inject(inject, carry, 0))
                    carry = norm_thunks(ic, pair, o)

            # ---- tail: flush the last PV, final norms, last output proj;
            # a few warm matmuls keep the PE at full clock through the
            # normalization chain ----
            flush_pv()
            tail_ps = mm_psum.tile([128, 512], f32, tag="qs", name="tailps")
            with nc.named_scope("warmup"):
                for _ in range(4):
                    nc.tensor.matmul(tail_ps[:], wsrc[:, 0:128], wsrc[:],
                                     start=True, stop=True)
            for t in carry:
                t()
            for c in range(4):
                outproj_chunk(12 + c)

    nc.compile()
    return nc


def _get_nc():
    if "nc" not in _CACHED:
        _CACHED["nc"] = _build()
    return _CACHED["nc"]


def kernel(x, w_qkv, w_out, b_out):
    import concourse.mybir as mybir
    from concourse.bass_utils import run_bass_kernel_spmd

    bf16 = mybir.dt.np(mybir.dt.bfloat16)

    x = np.asarray(x, dtype=np.float32)
    w_qkv = np.asarray(w_qkv, dtype=np.float32)
    w_out = np.asarray(w_out, dtype=np.float32)
    b_out = np.asarray(b_out, dtype=np.float32)

    in_maps = []
    for c in range(N_CORES):
        bi, hg = c // 2, c % 2
        f0 = hg * FEAT
        wq = w_qkv[:, f0:f0 + FEAT]
        wk = w_qkv[:, INNER + f0:INNER + f0 + FEAT]
        wvs = w_qkv[:, 2 * INNER + f0:2 * INNER + f0 + FEAT]
        in_maps.append({
            "xT": np.ascontiguousarray(x[bi].T).astype(bf16),
            "wqk": np.ascontiguousarray(
                np.concatenate([wq, wk], axis=1)).astype(bf16),
            "wv": np.ascontiguousarray(wvs).astype(bf16),
            "w_out": np.ascontiguousarray(w_out[f0:f0 + FEAT, :]).astype(bf16),
        })

    nc = _get_nc()
    res = run_bass_kernel_spmd(nc, in_maps, list(range(N_CORES)))

    outa = np.empty((B, N, DIM), dtype=np.float32)
    for bi in range(B):
        outa[bi] = (res.results[2 * bi]["out"]
                    + res.results[2 * bi + 1]["out"] + b_out)
    return outa


# revision 8
# speedup vs baseline: 1.0157x; 1.0157x over previous
"""Distributed Trainium2 kernel for the AttentionBlock problem.

Full inputs:
  x     [4, 2048, 512] f32
  w_qkv [512, 1536]    f32   (columns: q | k | v, each 512 wide)
  w_out [512, 512]     f32
  b_out [512]          f32

Sharding over 8 cores: core c handles batch (c // 2) and head-group
(c % 2) -> 4 heads of 64 dims each (feature slice of 256 per section).
Each core computes a partial output projection (its 4 heads' contribution
to out = attn @ w_out); the host sums the two partials per batch and adds
the bias.

Per-core dataflow (bf16 matmuls, all intermediates in SBUF; x and the
weights are converted to bf16 on the host so no on-device casts and half
the input DMA traffic):
  xT [512, 2048] -> qT,kT [256+256, 2048] (transposed layout)
                 -> v natural [2048, 4*65] (65th col per head = ones,
                    which makes the softmax denominator fall out of the
                    PV matmul as output row 64 for free: the PV stream
                    time is N cycles regardless of the stationary width)
  per (query-block of 512, head-pair) block, 16 j-steps:
    S^T[j 128, i 512x2] = kT.T @ qT   (two heads in concurrent 64-row
                                       PE tiles -> one [128,1024] PSUM)
    P^T = exp(S^T * 0.125)            (ScalarE -> SBUF bf16)
    outT[f 65, i 512] += v'.T @ P^T   (j-major PSUM accumulation)
    attnT[f, i] = outT[0:64] * (1/l)  (DVE recip from PSUM + gpsimd
                                       broadcast + DVE mul)
  out_partial[i 128, 512] = attnT.T @ w_out_shard -> DRAM

Schedule: ScalarE (128 exps of [128,1024], ~136us busy) is the critical
engine; everything else hides under it. Key tricks:
  - PV(j) is issued one slot late so its matmuls never wait on the
    just-finished exp (no PE pipe-refill stall).
  - The remaining q/k/v projections and the output projections are
    injected between attention slots where the PE has slack; the first
    two blocks are interleaved in j-halves (b0a, b1a, b0b, b1b) to
    spread the v-projection injections over 32 slots.
  - Normalizations run overlapped with the next block; the tail after
    the last exp is only PV flush + 2 norms + 4 output-projection
    chunks.
  - A few warmup matmuls run during the DMA ramp so the PE HAM clock
    gate is already at full rate when the real matmuls arrive.
"""

import sys

if "/opt/trn_rl_repo" not in sys.path:
    sys.path.insert(0, "/opt/trn_rl_repo")

import numpy as np

DIM = 512
HEADS = 8
DIM_HEAD = 64
INNER = 512
B, N = 4, 2048
N_CORES = 8
HEADS_PER_CORE = 4
FEAT = HEADS_PER_CORE * DIM_HEAD  # 256 features per core per section
SCALE = DIM_HEAD ** -0.5  # 0.125

N_JB = N // 128  # 16 j-blocks
N_WARMUP_MM = 8

_CACHED = {}


def _build():
    import concourse.mybir as mybir
    import concourse.tile as tile
    from concourse import bacc

    f32 = mybir.dt.float32
    bf16 = mybir.dt.bfloat16
    EXP = mybir.ActivationFunctionType.Exp
    MUL = mybir.AluOpType.mult

    nc = bacc.Bacc("TRN2", target_bir_lowering=False, debug=False,
                   num_devices=N_CORES)

    xT = nc.declare_dram_parameter("xT", [DIM, N], bf16, isOutput=False)
    wqk = nc.declare_dram_parameter("wqk", [DIM, 2 * FEAT], bf16,
                                    isOutput=False)
    wv = nc.declare_dram_parameter("wv", [DIM, FEAT], bf16, isOutput=False)
    w_out = nc.declare_dram_parameter("w_out", [FEAT, DIM], bf16,
                                      isOutput=False)
    out = nc.declare_dram_parameter("out", [N, DIM], f32, isOutput=True)

    with tile.TileContext(nc) as tc:
        with (
            tc.tile_pool(name="xb", bufs=1) as xb_pool,
            tc.tile_pool(name="wq", bufs=1) as w_pool,
            tc.tile_pool(name="qkt", bufs=1) as qkt_pool,
            tc.tile_pool(name="vs", bufs=1) as v_pool,
            tc.tile_pool(name="pt", bufs=8) as pt_pool,
            tc.tile_pool(name="attnT", bufs=1) as attnT_pool,
            tc.tile_pool(name="scl", bufs=8) as scl_pool,
            tc.tile_pool(name="dout", bufs=4) as dout_pool,
            tc.tile_pool(name="warm", bufs=1) as warm_pool,
            tc.tile_pool(name="mm", bufs=2, space="PSUM") as mm_psum,
            tc.tile_pool(name="pv", bufs=4, space="PSUM") as pv_psum,
        ):
            # ---- PE warmup: keep the HAM clock gate busy during the DMA
            # ramp so real matmuls start at full rate ----
            wsrc = warm_pool.tile([128, 512], bf16, tag="wsrc", name="wsrc")
            nc.vector.memset(wsrc[:], 0.0)
            wps = mm_psum.tile([128, 512], f32, tag="qs", name="warmps")
            with nc.named_scope("warmup"):
                for _ in range(N_WARMUP_MM):
                    nc.tensor.matmul(wps[:], wsrc[:, 0:128], wsrc[:],
                                     start=True, stop=True)

            # ---- DMAs: many small transfers spread across the DMA queues
            # beat few large strided ones; issued in need order: x block 0,
            # wqk, wv, rest of x, wo.  Logical layouts keep the [p, k, f]
            # indexing (k = contraction-block in dim 1). ----
            xb = [None] * 4  # [n] -> [128, 4, 512] bf16 (k in dim 1)

            def dma_x(n):
                t = xb_pool.tile([128, 4, 512], bf16, tag=f"xb{n}",
                                 name=f"xb{n}")
                for k in range(4):
                    nc.sync.dma_start(
                        out=t[:, k, :],
                        in_=xT[k * 128:(k + 1) * 128, n * 512:(n + 1) * 512])
                xb[n] = t

            dma_x(0)
            wqk_t = w_pool.tile([128, 4, 512], bf16, tag="wqk", name="wqkb")
            for k in range(4):
                nc.sync.dma_start(out=wqk_t[:, k, :],
                                  in_=wqk[k * 128:(k + 1) * 128, :])
            wv_t = w_pool.tile([128, 4, FEAT], bf16, tag="wv", name="wvb")
            for k in range(4):
                nc.sync.dma_start(out=wv_t[:, k, :],
                                  in_=wv[k * 128:(k + 1) * 128, :])
            dma_x(1)
            dma_x(2)
            dma_x(3)
            wo = w_pool.tile([128, 2, DIM], bf16, tag="wo", name="wob")
            for k in range(2):
                nc.sync.dma_start(out=wo[:, k, :],
                                  in_=w_out[k * 128:(k + 1) * 128, :])

            # ---- persistent SBUF tiles ----
            qkt = [qkt_pool.tile([128, N], bf16, tag=f"qkt{m}", name=f"qkt{m}")
                   for m in range(4)]
            vt = [v_pool.tile([128, 4 * 65], bf16, tag=f"v{j}", name=f"v{j}")
                  for j in range(N_JB)]
            attnT = [attnT_pool.tile([128, N], bf16, tag=f"aT{t}",
                                     name=f"aT{t}")
                     for t in range(2)]

            def proj_qk(m, n):
                ncol = slice(n * 512, (n + 1) * 512)
                with nc.named_scope("proj"):
                    ps = mm_psum.tile([128, 512], f32, tag="qs", name="psb")
                    for k in range(4):
                        nc.tensor.matmul(
                            ps[:],
                            wqk_t[:, k, m * 128:(m + 1) * 128],
                            xb[n][:, k, :],
                            start=(k == 0), stop=(k == 3),
                        )
                    nc.vector.tensor_copy(qkt[m][:, ncol], ps[:])

            def proj_v(j):
                n, jj = j // 4, j % 4
                with nc.named_scope("proj"):
                    ps = mm_psum.tile([128, 256], f32, tag="qs", name="psv")
                    for k in range(4):
                        nc.tensor.matmul(
                            ps[:],
                            xb[n][:, k, jj * 128:(jj + 1) * 128],
                            wv_t[:, k, :],
                            start=(k == 0), stop=(k == 3),
                        )
                    v_view = vt[j].rearrange("p (h f) -> p h f", h=4)
                    nc.vector.tensor_copy(
                        v_view[:, :, 0:64], ps.rearrange("p (h f) -> p h f", h=4)
                    )
                    nc.vector.memset(v_view[:, :, 64:65], 1.0)

            def normalize(pair, hh, ic_ps, i0):
                with nc.named_scope("norm"):
                    # copy the denominator row to SBUF first: a PSUM-sourced
                    # reciprocal_approx_fast reads the wrong partition
                    lrow = scl_pool.tile([1, 512], f32, tag="lrow", name="lrow")
                    nc.vector.tensor_copy(lrow[:], ic_ps[64:65, :])
                    rl = scl_pool.tile([1, 512], f32, tag="rl", name="rl")
                    nc.vector.reciprocal_approx_fast(rl[:], lrow[:])
                    rlb = scl_pool.tile([64, 512], f32, tag="rlb", name="rlb")
                    nc.gpsimd.partition_broadcast(rlb[:], rl[:])
                    nc.vector.tensor_tensor(
                        attnT[pair][hh * 64:(hh + 1) * 64, i0:i0 + 512],
                        ic_ps[0:64, :], rlb[:], MUL,
                    )

            def outproj_chunk(chunk):
                with nc.named_scope("outproj"):
                    ps = mm_psum.tile([128, 512], f32, tag="qs", name="psd")
                    for t in range(2):
                        nc.tensor.matmul(
                            ps[:],
                            attnT[t][:, chunk * 128:(chunk + 1) * 128],
                            wo[:, t, :],
                            start=(t == 0), stop=(t == 1),
                        )
                    ot = dout_pool.tile([128, 512], f32, tag="ot", name="ot")
                    nc.vector.tensor_copy(ot[:], ps[:])
                    nc.sync.dma_start(out=out[chunk * 128:(chunk + 1) * 128, :],
                                      in_=ot[:])

            # pending PV thunk: PV(j) is issued one attention slot later so
            # its matmuls never wait on the exp that produced their P tile.
            pend = [None]

            def flush_pv():
                if pend[0] is not None:
                    pend[0]()
                    pend[0] = None

            def attn_phase(ic, pair, jlo, jhi, outp, inject):
                qt, kt = qkt[pair], qkt[2 + pair]
                i0 = ic * 512
                for j in range(jlo, jhi):
                    with nc.named_scope("attnS"):
                        qs = mm_psum.tile([128, 1024], f32, tag="qs",
                                          name="qs")
                        for hh in range(2):
                            nc.tensor.matmul(
                                qs[:, hh * 512:(hh + 1) * 512],
                                kt[hh * 64:(hh + 1) * 64,
                                   j * 128:(j + 1) * 128],
                                qt[hh * 64:(hh + 1) * 64, i0:i0 + 512],
                                start=True, stop=True,
                            )
                    with nc.named_scope("exp"):
                        p = pt_pool.tile([128, 1024], bf16, tag="pt",
                                         name="ptile")
                        nc.scalar.activation(p[:], qs[:], EXP, scale=SCALE)
                    flush_pv()

                    def pv(p=p, j=j):
                        with nc.named_scope("pv"):
                            for hh in range(2):
                                h = 2 * pair + hh
                                nc.tensor.matmul(
                                    outp[hh][0:65, :],
                                    vt[j][:, h * 65:(h + 1) * 65],
                                    p[:, hh * 512:(hh + 1) * 512],
                                    start=(j == 0), stop=(j == N_JB - 1),
                                )
                    pend[0] = pv
                    for thunk in inject.get(j, ()):
                        thunk()

            def new_outp(ic, pair):
                return [pv_psum.tile([128, 512], f32, tag="pv",
                                     name=f"o{ic}{pair}{hh}")
                        for hh in range(2)]

            def norm_thunks(ic, pair, outp):
                return [
                    (lambda hh=hh: normalize(pair, hh, outp[hh], ic * 512))
                    for hh in range(2)
                ]

            def merge_inject(inject, extra, jlo):
                inj = {j: list(v) for j, v in inject.items()}
                for i, t in enumerate(extra):
                    inj.setdefault(jlo + i, []).insert(0, t)
                return inj

            P = proj_qk
            V = proj_v

            # ---- prologue: minimum work before the first exp ----
            P(0, 0)
            P(2, 0)
            V(0)
            V(1)

            # ---- ic0: the two blocks interleaved in j-halves so the
            # remaining projections spread over 32 slots ----
            o00 = new_outp(0, 0)
            o01 = new_outp(0, 1)
            attn_phase(0, 0, 0, 8, o00, {
                1: [lambda: V(2)],
                2: [lambda: V(3)],
                3: [lambda: P(2, 1), lambda: V(4)],
                4: [lambda: V(5)],
                5: [lambda: V(6)],
                6: [lambda: V(7), lambda: P(1, 0)],
                7: [lambda: P(3, 0)],
            })
            attn_phase(0, 1, 0, 8, o01, {
                2: [lambda: P(2, 2)],
                3: [lambda: P(3, 1)],
                4: [lambda: V(8)],
                5: [lambda: V(9)],
                6: [lambda: V(10)],
                7: [lambda: V(11)],
            })
            attn_phase(0, 0, 8, 16, o00, {
                10: [lambda: P(2, 3)],
                11: [lambda: P(3, 2)],
                12: [lambda: V(12)],
                13: [lambda: V(13)],
                14: [lambda: V(14)],
                15: [lambda: V(15)],
            })
            attn_phase(0, 1, 8, 16, o01, merge_inject({
                10: [lambda: P(3, 3)],
                11: [lambda: P(0, 1)],
                12: [lambda: P(1, 1)],
                13: [lambda: P(0, 2)],
                14: [lambda: P(1, 2)],
                15: [lambda: P(0, 3)],
            }, norm_thunks(0, 0, o00), 8))
            carry = norm_thunks(0, 1, o01)

            # ---- ic1..ic3: plain blocks; the previous block's norms are
            # injected into the first slots (after the slot-0 PV flush) and
            # the previous ic's output projections once its norms landed ----
            for ic in range(1, 4):
                for pair in range(2):
                    o = new_outp(ic, pair)
                    inject = {}
                    if pair == 1:
                        inject = {2 + 4 * c: [
                            (lambda c=c: outproj_chunk(4 * (ic - 1) + c))]
                            for c in range(4)}
                    elif ic == 1:
                        inject = {2: [lambda: P(1, 3)]}
                    attn_phase(ic, pair, 0, N_JB, o,
                               merge_inject(inject, carry, 0))
                    carry = norm_thunks(ic, pair, o)

            # ---- tail: flush the last PV, final norms, last output proj;
            # a few warm matmuls keep the PE at full clock through the
            # normalization chain ----
            flush_pv()
            tail_ps = mm_psum.tile([128, 512], f32, tag="qs", name="tailps")
            with nc.named_scope("warmup"):
                for _ in range(16):
                    nc.tensor.matmul(tail_ps[:], wsrc[:, 0:128], wsrc[:],
                                     start=True, stop=True)
            for t in carry:
                t()
            for c in range(4):
                outproj_chunk(12 + c)

    nc.compile()
    return nc


def _get_nc():
    if "nc" not in _CACHED:
        _CACHED["nc"] = _build()
    return _CACHED["nc"]


def kernel(x, w_qkv, w_out, b_out):
    import concourse.mybir as mybir
    from concourse.bass_utils import run_bass_kernel_spmd

    bf16 = mybir.dt.np(mybir.dt.bfloat16)

    x = np.asarray(x, dtype=np.float32)
    w_qkv = np.asarray(w_qkv, dtype=np.float32)
    w_out = np.asarray(w_out, dtype=np.float32)
    b_out = np.asarray(b_out, dtype=np.float32)

    in_maps = []
    for c in range(N_CORES):
        bi, hg = c // 2, c % 2
        f0 = hg * FEAT
        wq = w_qkv[:, f0:f0 + FEAT]
        wk = w_qkv[:, INNER + f0:INNER + f0 + FEAT]
        wvs = w_qkv[:, 2 * INNER + f0:2 * INNER + f0 + FEAT]
        in_maps.append({
            "xT": np.ascontiguousarray(x[bi].T).astype(bf16),
            "wqk": np.ascontiguousarray(
                np.concatenate([wq, wk], axis=1)).astype(bf16),
            "wv": np.ascontiguousarray(wvs).astype(bf16),
            "w_out": np.ascontiguousarray(w_out[f0:f0 + FEAT, :]).astype(bf16),
        })

    nc = _get_nc()
    res = run_bass_kernel_spmd(nc, in_maps, list(range(N_CORES)))

    outa = np.empty((B, N, DIM), dtype=np.float32)
    for bi in range(B):
        outa[bi] = (res.results[2 * bi]["out"]
                    + res.results[2 * bi + 1]["out"] + b_out)
    return outa
